# revision 38
# baseline (speedup 1.0000x reference)
"""Trainium2 Bass kernel for nn_LocalWLGNN (gnn_message_passing).

Reference computation (per layer l, x: [N, D]):
    out = (1+eps) * x
    for hop in range(H):
        agg = segment_sum(x[scatter_idx[hop]], node_idx[hop], N)
        out += relu((x + agg) @ w1[l,hop]) @ w2[l,hop]
    x = out

Sharding: 8 cores, core k owns destination nodes [k*N/8, (k+1)*N/8).

Design (bf16 data path):
  - Node features live in DRAM as bf16.  Per (core, hop) the edge list is
    split into 2 passes by source-node window (so row indices fit the
    int16 dma_gather contract); within a pass the destinations are
    degree-sorted so round j (= j-th in-edge of every destination) covers
    a contiguous position prefix.
  - dma_gather (SWDGE) fetches rows from DRAM in large capped calls that
    ignore round boundaries; DVE accumulates per-(call, round) segments
    into a pass-ordered agg tile (bf16).
  - The pass->canonical permutation is done by an SBUF-source transposed
    dma_gather which simultaneously transposes to feature-major: one call
    yields aggT [128=feat, nshp=node] directly (replacing a CCE
    scatter_add + 49 PE transposes per hop).
  - xpaT = xkT + aggT(pass0) + aggT(pass1) on DVE; 2-layer MLP in bf16 on
    PE (fp32 PSUM); hop outputs accumulate into feature-major f32 outaccT.
  - Between layers, xkT/outaccT are recomputed feature-major with plain
    copies (no transposes); only the DRAM outputs (AllGather input shard,
    final out) are back-transposed to node-major.
  - An 8-core bf16 AllGather republishes node features between layers.

Performance: the kernel is bound by SWDGE descriptor processing — every
per-edge random row move (DRAM gather or SBUF permute) costs ~8 ns PER
QUEUE regardless of element size or source (DRAM vs SBUF).  The shipped
config spreads the ~470k descriptors/core over all 4 SWDGE queues
(queue = DMASW sem lane % 4, rewritten post-scheduling so each lane
stays single-queue; this is REQUIRED for correctness — emit-time
queue_num assignment corrupts) with cap=2560 gather calls and
gt=12/ix=2/xk=2/xpa=2 pool buffers.  Two further structural wins:
(1) SOFTWARE-PIPELINED EMISSION — the permute's descriptor generation
(Pool engine, in-order) waits on its pass's last DVE add, so emitting
it right after that pass stalls every later-emitted gather generation;
deferring each unit's permute/xpa/MLP until after the NEXT unit's
gathers are emitted saved ~0.28 ms (pdepth=1; pdepth=2 is slightly
worse).  (2) outaccT in BF16 (oacc_bf16=True) — frees 3.2 MB SBUF
(spent on gt=12, which only helps ONCE emission is pipelined) at
rel-err 7.8e-3 vs 7.1e-3 in f32.  Measured (For_i repeat-delta):
gathers alone 0.93 ms (≈ the 4-queue descriptor roofline); full body
1.35 ms no-collective; async full 1.80-1.85 ms.  History: 4.03 ms
single-queue, 1.90 ms 4-queue unpipelined.  Runs are BIMODAL (~1.72 vs ~2.0 ms for identical builds —
physical-core placement); compare configs with multiple runs.
Dead ends measured: gpsimd ap_gather / scatter_add run ~10x the cost
model (~75-90 us per 6k-col op) so matmul-free scatter designs lose;
single_packet=True wedges the device (NRT_EXEC_UNIT_UNRECOVERABLE);
psplit>1 (splitting one permute into concurrent SBUF-source sub-
gathers across queues) is ~70 us FASTER but CORRUPTS results
(rel-err 0.56-0.79) — and rsplit=True (second sub-call reads a DVE
replica of agg, i.e. DIFFERENT source tiles) corrupts identically, so
the race is GLOBAL to concurrent SBUF-source transposed gathers
(shared ring/scratch state across queues), not per-tile.  The shipped
kernel is structurally safe only because consecutive permutes are
separated by a full unit's gather drain (~78 us > 50 us permute
drain); keep psplit=1/rsplit=False and preserve that spacing;
cap=1024/2048 lose to per-call overhead at gt-equivalent SBUF;
aggT=3 or ix=3 (traded against gt) lose ~50-90 us;
non-%128 gather num_idxs (16-granular round padding, to trim ~17k
zero-row descriptors) CORRUPTS results and saves nothing — keep
rounds 128-padded;
SBUF is at the 208 KiB/partition limit (xpa=3 overflows); gathering
round-0 STRAIGHT into agg serializes behind the agg tile's 2-buf WAR
window (+0.2 ms); round-0 copies on the Act engine thrash the
activation table against the MLP's Relu (+0.2 ms); an IN-PLACE
(1+eps) DVE scale of outaccT at the layer boundary (to drop oacc to
1 buf) CORRUPTS results — the Act xkT-copy / DVE in-place-write WAR
ordering does not hold, keep the two-buffer copy.
"""

import numpy as np


# ---------------------------------------------------------------- config

def make_cfg(N, D, E, H, L, ncores, cap, wlo=None):
    """The global node layout splits each core's shard into an H0-row half-A
    and an H1-row half-B; xg/xg2 hold [zeros(2), all cores' half-As, all
    cores' half-Bs, zeros(2)].  This makes each half an independent
    contiguous AllGather AND keeps both int16 gather windows under 32768
    rows.  `wlo` is derived (block-A end); the kwarg is ignored."""
    nsh = N // ncores
    nshp = -(-nsh // 128) * 128          # padded positions per core
    H0 = min(nsh - 128, max(128, ((nsh // 2 + 127) // 128) * 128))
    H1 = nsh - H0
    wlo = 2 + ncores * H0
    assert wlo <= 32768 and (N + 4) - wlo <= 32768
    assert cap % 128 == 0
    return dict(N=N, D=D, E=E, H=H, L=L, ncores=ncores, nsh=nsh, nshp=nshp,
                H0=H0, H1=H1, wlo=wlo, cap=cap)


FULL_CFG = make_cfg(N=50000, D=128, E=500000, H=3, L=2, ncores=8, cap=2560)
FULL_BUFS = {"gt": 12, "ix": 2, "xk": 2, "xpa": 2}
FULL_NQUEUES = 4


# ----------------------------------------------------- host preprocessing

def build_schedule(scatter_idx, node_idx, cfg):
    """Bucket edges per (core, hop, pass), degree-sort, build rounds.

    Returns:
      calls: static per-(hop, pass) list of gather calls, identical for all
             cores: [(n, [(agg_pos, gt_off, seg_len, is_copy), ...]), ...]
             with n, agg_pos, gt_off, seg_len all multiples of 128.
      gidx:  [ncores, 128, gcols] int16 gather index data (16-wrapped, 8x
             partition-replicated).
      pidx:  [ncores, 128, H*2*(nshp//16)] int16 permute (SBUF gather) idxs.
      seg_cols: per-(hop, pass) (col_off, cols) into gidx free dim.
    """
    N, H, nc_, nsh, nshp = cfg["N"], cfg["H"], cfg["ncores"], cfg["nsh"], cfg["nshp"]
    H0, H1, wlo, cap = cfg["H0"], cfg["H1"], cfg["wlo"], cfg["cap"]
    zlo = 0                      # xg row 0 is zeros (lo-window pad index)
    zhi = (N + 3) - wlo          # xg row N+3 is zeros, local to hi window

    rounds_khp = {}              # (h, p, k) -> list of per-round idx arrays
    perm_khp = {}                # (h, p, k) -> canonical d -> pass position
    maxpref = {}                 # (h, p) -> list of per-round max prefix
    for h in range(H):
        src_h = np.asarray(scatter_idx[h]).astype(np.int64)
        dst_h = np.asarray(node_idx[h]).astype(np.int64)
        core_of = dst_h // nsh
        for k in range(nc_):
            m = core_of == k
            src_k = src_h[m]
            dst_k = dst_h[m] - k * nsh
            kk = src_k // nsh             # owning core of each source
            ii = src_k - kk * nsh         # within-shard offset
            for p in range(2):            # 0 = half-A window, 1 = half-B
                if p == 0:
                    mm = ii < H0
                    ps = 2 + kk[mm] * H0 + ii[mm]         # window-local row
                else:
                    mm = ii >= H0
                    ps = kk[mm] * H1 + (ii[mm] - H0)
                pd = dst_k[mm]
                deg = np.bincount(pd, minlength=nsh)
                order = np.argsort(-deg, kind="stable")
                pos = np.empty(nsh, np.int64)
                pos[order] = np.arange(nsh)
                perm = np.arange(nshp, dtype=np.int64)
                perm[:nsh] = pos
                perm_khp[(h, p, k)] = perm
                key = pos[pd]
                so = np.argsort(key, kind="stable")
                ps_s = ps[so]
                key_s = key[so]
                rank = np.arange(len(key_s)) - np.searchsorted(key_s, key_s)
                maxdeg = int(deg.max()) if len(deg) else 0
                rlist = [ps_s[rank == j] for j in range(maxdeg)]
                rounds_khp[(h, p, k)] = rlist
                mp = maxpref.setdefault((h, p), [])
                for j, r in enumerate(rlist):
                    if j < len(mp):
                        mp[j] = max(mp[j], len(r))
                    else:
                        mp.append(len(r))

    # Static structure per (h, p): the index blob is the concatenation of
    # per-round arrays, round 0 padded to nshp (covers deg-0 positions with
    # zero rows), round j>=1 padded to ceil(max_prefix/128)*128.  Round-0
    # calls are emitted separately (pure position-aligned copies via gt
    # staging + one wide DVE copy each); the remaining blob is chunked by
    # `cap` with (agg position, gt offset, length) add segments.
    calls = {}
    blob_widths = {}
    for h in range(H):
        for p in range(2):
            mp = maxpref.get((h, p), [])
            widths = [nshp]
            for j in range(1, len(mp)):
                if mp[j]:
                    widths.append(-(-mp[j] // 128) * 128)
            blob_widths[(h, p)] = widths
            starts = np.cumsum([0] + widths)
            total = int(starts[-1])
            r0 = []
            o = 0
            while o < nshp:
                n = min(cap, nshp - o)
                r0.append((n, o))
                o += n
            cl = []
            while o < total:
                n = min(cap, total - o)
                segs = []
                for j, w in enumerate(widths):
                    if j == 0:
                        continue
                    a = max(o, int(starts[j]))
                    b = min(o + n, int(starts[j]) + w)
                    if a < b:
                        segs.append((a - int(starts[j]), a - o, b - a))
                cl.append((n, segs))
                o += n
            calls[(h, p)] = (r0, cl)

    # gather idx blobs
    seg_cols = {}
    col = 0
    for h in range(H):
        for p in range(2):
            ncols = sum(blob_widths[(h, p)]) // 16
            seg_cols[(h, p)] = (col, ncols)
            col += ncols
    gcols = col
    gidx = np.zeros((nc_, 128, gcols), np.int16)
    for k in range(nc_):
        for h in range(H):
            for p in range(2):
                zpad = zlo if p == 0 else zhi
                rlist = rounds_khp[(h, p, k)]
                widths = blob_widths[(h, p)]
                padded = []
                for j, w in enumerate(widths):
                    v = np.full(w, zpad, np.int64)
                    if j < len(rlist):
                        v[: len(rlist[j])] = rlist[j]
                    padded.append(v)
                blob = np.concatenate(padded)
                c0, ncols = seg_cols[(h, p)]
                assert blob.size == ncols * 16, (blob.size, ncols * 16)
                wrapped = blob.reshape(ncols, 16).T.astype(np.int16)
                gidx[k, :, c0:c0 + ncols] = np.tile(wrapped, (8, 1))

    # permute idx blobs: per (h, p) a column range of width nshp//16
    pcols = nshp // 16
    pidx = np.zeros((nc_, 128, H * 2 * pcols), np.int16)
    for k in range(nc_):
        for h in range(H):
            for p in range(2):
                v = perm_khp[(h, p, k)]
                wrapped = v.reshape(pcols, 16).T.astype(np.int16)
                c0 = (h * 2 + p) * pcols
                pidx[k, :, c0:c0 + pcols] = np.tile(wrapped, (8, 1))

    return calls, gidx, pidx, seg_cols


# ------------------------------------------------------- device program

def build_program(cfg, calls, seg_cols, repeat=1, no_collective=False,
                  loop_repeat=None, skip_mlp=False, gather_only=False,
                  skip_adds=False, single_packet=False, nqueues=1,
                  psplit=1, bufs=None, oacc_bf16=False, pdepth=1,
                  rsplit=False):
    import concourse.bacc as bacc
    import concourse.tile as tile
    from concourse import bass, mybir
    from concourse import library_config

    N, D, H, L = cfg["N"], cfg["D"], cfg["H"], cfg["L"]
    nsh, nshp, wlo = cfg["nsh"], cfg["nshp"], cfg["wlo"]
    nc_cores = cfg["ncores"]
    f32 = mybir.dt.float32
    bf16 = mybir.dt.bfloat16
    i16 = mybir.dt.int16
    CH = nshp // 128                      # position chunks per core
    pcols = nshp // 16
    gcols = max(c0 + nc for (c0, nc) in seg_cols.values())

    nc = bacc.Bacc("TRN2", target_bir_lowering=False, debug=False,
                   num_devices=cfg["ncores"], num_swdge_queues=nqueues)

    nhi = (N + 4) - wlo
    xga_in = nc.dram_tensor("xga", [wlo, D], bf16, kind="ExternalInput")
    xgb_in = nc.dram_tensor("xgb", [nhi, D], bf16, kind="ExternalInput")
    gidx_t = nc.dram_tensor("gidx", [128, gcols], i16, kind="ExternalInput")
    pidx_t = nc.dram_tensor("pidx", [128, H * 2 * pcols], i16,
                            kind="ExternalInput")
    xkT_t = nc.dram_tensor("xkT", [128, nshp], bf16, kind="ExternalInput")
    ident_t = nc.dram_tensor("ident", [128, 128], f32, kind="ExternalInput")
    w1_t = nc.dram_tensor("w1f", [L * H * D, D], bf16, kind="ExternalInput")
    w2_t = nc.dram_tensor("w2f", [L * H * D, D], bf16, kind="ExternalInput")
    eps1_t = nc.dram_tensor("eps1", [128, 1], f32, kind="ExternalInput")
    out_t = nc.dram_tensor("out", [nshp, D], f32, kind="ExternalOutput")

    # Separate window-A / window-B republish tensors: window-A gathers of the
    # next layer then depend only on the half-A AllGather, never on half-B.
    xg2a = nc.dram_tensor("xg2a", [wlo, D], bf16, addr_space="Shared")
    xg2b = nc.dram_tensor("xg2b", [nhi, D], bf16, addr_space="Shared")
    agin = nc.dram_tensor("agin", [nshp, D], bf16)         # AG input shard

    bufs = bufs or {}
    with tile.TileContext(nc) as tc:
        gt_bufs = bufs.get("gt", 3 if nqueues == 1 else 2)
        xk_bufs = bufs.get("xk", 2 if nqueues == 1 else 1)
        with (
            tc.tile_pool(name="persist", bufs=1) as pp,
            tc.tile_pool(name="xkT", bufs=xk_bufs) as xkp,
            tc.tile_pool(name="oacc", bufs=bufs.get("oacc", 2)) as oap,
            tc.tile_pool(name="aggp", bufs=bufs.get("agg", 2)) as aggp,
            tc.tile_pool(name="aggT", bufs=bufs.get("aggT", 2)) as aggTp,
            tc.tile_pool(name="agg2", bufs=1) as agg2p,
            tc.tile_pool(name="xpaT",
                         bufs=bufs.get("xpa", 2 if nqueues == 1 else 1)
                         ) as xpap,
            tc.tile_pool(name="gt", bufs=gt_bufs) as gtp,
            tc.tile_pool(name="ix",
                         bufs=bufs.get("ix", 2 if nqueues == 1 else 1)) as ixp,
            tc.tile_pool(name="r1p", bufs=2) as r1p,
            tc.tile_pool(name="stg", bufs=2) as stgp,
            tc.tile_pool(name="ps", bufs=2, space="PSUM") as psp,
            tc.tile_pool(name="ps2", bufs=2, space="PSUM") as ps2p,
            tc.tile_pool(name="pst", bufs=2, space="PSUM") as pstp,
        ):
            nc.gpsimd.load_library(library_config.mlp)
            _regs = {}

            def nreg(v):
                if v not in _regs:
                    _regs[v] = nc.gpsimd.to_reg(v)
                return _regs[v]

            def next_q():
                # queue_num is rewritten after scheduling (see below) so that
                # each DMASW sem lane maps to exactly one SWDGE queue; the
                # emit-time value is a placeholder.
                return 0

            oat = bf16 if oacc_bf16 else f32
            ident_f32 = pp.tile([128, 128], f32, tag="ident")
            nc.sync.dma_start(ident_f32[:], ident_t[:, :])
            if oacc_bf16:
                # transpose is matmul(out, in_, identity): identity must
                # match outaccT's dtype
                ident = pp.tile([128, 128], bf16, tag="ident16")
                nc.vector.tensor_copy(ident[:], ident_f32[:])
            else:
                ident = ident_f32
            eps1 = pp.tile([128, 1], f32, tag="eps1")
            nc.sync.dma_start(eps1[:], eps1_t[:, :])
            pidx_sb = pp.tile([128, H * 2 * pcols], i16, tag="pidx")
            nc.sync.dma_start(pidx_sb[:], pidx_t[:, :])
            wtiles = {}
            for l in range(L):
                for h in range(H):
                    wt1 = pp.tile([128, D], bf16, tag=f"w1_{l}_{h}")
                    wt2 = pp.tile([128, D], bf16, tag=f"w2_{l}_{h}")
                    lh = l * H + h
                    nc.sync.dma_start(wt1[:], w1_t[lh * D:(lh + 1) * D, :])
                    nc.sync.dma_start(wt2[:], w2_t[lh * D:(lh + 1) * D, :])
                    wtiles[(l, h)] = (wt1, wt2)

            # zero the pad rows (xg2a rows 0,1; xg2b last 2 rows)
            ztile = pp.tile([2, D], bf16, tag="zz")
            nc.vector.memset(ztile[:], 0.0)
            nc.sync.dma_start(xg2a[0:2, :], ztile[:])
            nc.sync.dma_start(xg2b[nhi - 2:nhi, :], ztile[:])

            import contextlib
            rep_ctx = (tc.For_i(0, loop_repeat, 1) if loop_repeat
                       else contextlib.nullcontext())
            with rep_ctx:
             for rep in range(repeat):
              # xkT = x_k^T (feature-major bf16), outaccT = (1+eps)*x_k^T
              xkT = xkp.tile([128, nshp], bf16, tag="xkT")
              nc.sync.dma_start(xkT[:], xkT_t[:, :])
              outaccT = oap.tile([128, nshp], oat, tag="oacc")
              if not (gather_only or skip_adds):
                  o = 0
                  while o < nshp:
                      w = min(512, nshp - o)
                      nc.scalar.activation(
                          outaccT[:, o:o + w], xkT[:, o:o + w],
                          mybir.ActivationFunctionType.Copy,
                          scale=eps1[:, 0:1])
                      o += w

              for l in range(L):
                if l == 0:
                    win = {0: xga_in[:, :], 1: xgb_in[:, :]}
                else:
                    win = {0: xg2a[:, :], 1: xg2b[:, :]}

                # Software-pipelined emission: the permute's descriptor
                # generation (Pool engine, in-order) waits on its pass's
                # last DVE add, so emitting it right after that pass's
                # gathers stalls every later-emitted gather generation
                # behind it (and the xpa-add similarly stalls the in-order
                # DVE stream).  Emit unit (h,p)'s gathers first, THEN the
                # PREVIOUS unit's permute + xpa-add + MLP — by the time
                # Pool reaches the deferred permute its adds have finished
                # under the current unit's gathers.
                xpaTs = {}

                def emit_unit(h, p):
                    c0, ncols = seg_cols[(h, p)]
                    iseg = ixp.tile([128, ncols], i16, tag="iseg")
                    nc.sync.dma_start(iseg[:], gidx_t[:, c0:c0 + ncols])
                    agg = aggp.tile([128, nshp], bf16, tag="agg")
                    r0_calls, rest_calls = calls[(h, p)]
                    icol = 0
                    # round 0 is a pure position-aligned copy: stage via gt
                    # (fast-recycling pool; direct-to-agg serializes behind
                    # the agg WAR window, Act copies thrash the act table)
                    for (n, po) in r0_calls:
                        gt = gtp.tile([128, cfg["cap"]], bf16, tag="gt")
                        nc.gpsimd.dma_gather(
                            gt[:, 0:n].rearrange("p (g e) -> p g e", e=D),
                            win[p],
                            iseg[:, icol:icol + n // 16],
                            n, nreg(n), D, single_packet=single_packet,
                            queue_num=next_q())
                        if not (gather_only or skip_adds):
                            nc.vector.tensor_copy(
                                agg[:, po:po + n], gt[:, 0:n])
                        icol += n // 16
                    for (n, segs) in rest_calls:
                        gt = gtp.tile([128, cfg["cap"]], bf16, tag="gt")
                        nc.gpsimd.dma_gather(
                            gt[:, 0:n].rearrange("p (g e) -> p g e", e=D),
                            win[p],
                            iseg[:, icol:icol + n // 16],
                            n, nreg(n), D, single_packet=single_packet,
                            queue_num=next_q())
                        if not (gather_only or skip_adds):
                            for (po, go, ln) in segs:
                                nc.vector.tensor_tensor(
                                    agg[:, po:po + ln],
                                    agg[:, po:po + ln],
                                    gt[:, go:go + ln],
                                    mybir.AluOpType.add)
                        icol += n // 16
                    return agg

                def finish_unit(h, p, agg):
                    # permute+transpose: aggT = agg[perm]^T (feat-major)
                    at = aggTp.tile([128, nshp], bf16, tag="aggT")
                    pc0 = (h * 2 + p) * pcols
                    if rsplit:
                        # split the permute across two queues WITHOUT the
                        # same-tile SBUF-source race: the second sub-call
                        # reads a DVE-copied replica of agg
                        agg2 = agg2p.tile([128, nshp], bf16, tag="agg2")
                        nc.vector.tensor_copy(agg2[:], agg[:])
                        half = (nshp // 256) * 128
                        nc.gpsimd.dma_gather(
                            at[:, 0:half].rearrange("p (o n) -> p o n", o=1),
                            agg[:],
                            pidx_sb[:, pc0:pc0 + half // 16],
                            half, nreg(half), D,
                            transpose=True, single_packet=single_packet,
                            sbuf_tokens_per_rank=128,
                            sbuf_free_dim_per_rank=D * 2,
                            queue_num=next_q())
                        nc.gpsimd.dma_gather(
                            at[:, half:nshp]
                              .rearrange("p (o n) -> p o n", o=1),
                            agg2[:],
                            pidx_sb[:, pc0 + half // 16:pc0 + pcols],
                            nshp - half, nreg(nshp - half), D,
                            transpose=True, single_packet=single_packet,
                            sbuf_tokens_per_rank=128,
                            sbuf_free_dim_per_rank=D * 2,
                            queue_num=next_q())
                        return _finish_tail(h, p, at)
                    po = 0
                    for s in range(psplit):
                        nsub = (nshp // 128 * (s + 1) // psplit
                                - nshp // 128 * s // psplit) * 128
                        if nsub == 0:
                            continue
                        nc.gpsimd.dma_gather(
                            at[:, po:po + nsub]
                              .rearrange("p (o n) -> p o n", o=1),
                            agg[:],
                            pidx_sb[:, pc0 + po // 16:
                                    pc0 + (po + nsub) // 16],
                            nsub, nreg(nsub), D,
                            transpose=True, single_packet=single_packet,
                            sbuf_tokens_per_rank=128,
                            sbuf_free_dim_per_rank=D * 2,
                            queue_num=next_q())
                        po += nsub
                    return _finish_tail(h, p, at)

                def _finish_tail(h, p, at):
                    if skip_mlp or skip_adds:
                        return
                    if p == 0:
                        xpaT = xpap.tile([128, nshp], bf16, tag="xpaT")
                        nc.vector.tensor_tensor(
                            xpaT[:], xkT[:], at[:], mybir.AluOpType.add)
                        xpaTs[h] = xpaT
                    else:
                        xpaT = xpaTs.pop(h)
                        nc.vector.tensor_tensor(
                            xpaT[:], xpaT[:], at[:], mybir.AluOpType.add)
                        # MLP: outaccT += relu(w1^T @ xpa^T chunks) via w2
                        wt1, wt2 = wtiles[(l, h)]
                        o = 0
                        while o < nshp:
                            w = min(512, nshp - o)
                            ps1 = psp.tile([128, 512], f32, tag="mm1")
                            nc.tensor.matmul(ps1[:, :w], wt1[:],
                                             xpaT[:, o:o + w],
                                             start=True, stop=True)
                            r1 = r1p.tile([128, 512], bf16, tag="r1")
                            nc.scalar.activation(
                                r1[:, :w], ps1[:, :w],
                                mybir.ActivationFunctionType.Relu)
                            ps2 = ps2p.tile([128, 512], f32, tag="mm2")
                            nc.tensor.matmul(ps2[:, :w], wt2[:], r1[:, :w],
                                             start=True, stop=True)
                            nc.vector.tensor_tensor(
                                outaccT[:, o:o + w], outaccT[:, o:o + w],
                                ps2[:, :w], mybir.AluOpType.add)
                            o += w

                pend = []
                for h in range(H):
                    for p in range(2):
                        pend.append((h, p, emit_unit(h, p)))
                        if len(pend) > pdepth and not gather_only:
                            finish_unit(*pend.pop(0))
                if not gather_only:
                    for u in pend:
                        finish_unit(*u)

                if gather_only or skip_mlp or skip_adds:
                    continue
                # back-transpose outaccT -> node-major rows for DRAM output
                dst_bf16 = l < L - 1
                tp = None
                for c in range(CH):
                    q = c % 4
                    if q == 0:
                        tp = pstp.tile([128, 512], oat, tag="tp")
                        stg = stgp.tile([128, 512], bf16 if dst_bf16 else f32,
                                        tag="stg")
                    nc.tensor.transpose(
                        tp[:, q * 128:(q + 1) * 128],
                        outaccT[:, c * 128:(c + 1) * 128], ident[:])
                    if q == 3 or c == CH - 1:
                        w = (q + 1) * 128
                        nc.scalar.activation(
                            stg[:, :w], tp[:, :w],
                            mybir.ActivationFunctionType.Copy)
                        dst = agin if dst_bf16 else out_t
                        nc.sync.dma_start(
                            dst.ap()[(c - q) * 128:(c - q) * 128 + w, :]
                               .rearrange("(c p) e -> p c e", p=128),
                            stg[:, :w].rearrange("p (c e) -> p c e", e=D))

                if l < L - 1:
                    if no_collective:
                        nc.sync.dma_start(xg2a[2:2 + nsh, :], agin[0:nsh, :])
                    else:
                        # Two half-shard AllGathers: half-A needs only the
                        # first H0/128 back-transpose chunks, so it launches
                        # while the rest of the back-transpose (and hop-2 MLP
                        # tail) still runs; next-layer half-A gathers need
                        # only AG-A (separate tensors make that dependency
                        # structural).
                        H0c = cfg["H0"]
                        nc.gpsimd.collective_compute(
                            "AllGather", mybir.AluOpType.bypass,
                            replica_groups=[list(range(nc_cores))],
                            ins=[agin[0:H0c, :]],
                            outs=[xg2a[2:2 + nc_cores * H0c, :]])
                        nc.gpsimd.collective_compute(
                            "AllGather", mybir.AluOpType.bypass,
                            replica_groups=[list(range(nc_cores))],
                            ins=[agin[H0c:nsh, :]],
                            outs=[xg2b[0:nc_cores * cfg["H1"], :]])
                    # next layer's xkT / outaccT from feature-major outaccT
                    xkT_new = xkp.tile([128, nshp], bf16, tag="xkT")
                    oacc_new = oap.tile([128, nshp], oat, tag="oacc")
                    o = 0
                    while o < nshp:
                        w = min(512, nshp - o)
                        nc.scalar.activation(
                            xkT_new[:, o:o + w], outaccT[:, o:o + w],
                            mybir.ActivationFunctionType.Copy)
                        nc.scalar.activation(
                            oacc_new[:, o:o + w], outaccT[:, o:o + w],
                            mybir.ActivationFunctionType.Copy,
                            scale=eps1[:, 0:1])
                        o += w
                    xkT = xkT_new
                    outaccT = oacc_new

    # Align SWDGE queue assignment with the scheduler's DMASW sem-lane
    # round-robin: a sem lane's semaphores are reused assuming in-order
    # completion, so every instruction on lane k must use the same queue.
    # queue = k % nqueues keeps lanes single-queue while spreading work.
    if nqueues > 1:
        from concourse.tile_scheduler import PROC_NAME_TO_IDX

        lane_q = {}
        for name, idx in PROC_NAME_TO_IDX.items():
            if name.startswith("DMASW"):
                lane_q[idx] = int(name[5:]) % nqueues
        from concourse import bass_isa

        for blk in nc.m.functions[0].blocks:
            for inst in blk.instructions:
                if isinstance(inst, (mybir.InstDMAGatherAnt,
                                     mybir.InstDMAScatterAddAnt)):
                    proc = inst.bass_scheduled_proc
                    if proc in lane_q:
                        inst.queue_num = lane_q[proc]

    nc.compile()
    return nc


# ------------------------------------------------------------- entry

def _prep_inputs(x, w1, w2, eps, scatter_idx, node_idx, cfg):
    import ml_dtypes

    bf = ml_dtypes.bfloat16
    N, D, H, L, nc_ = cfg["N"], cfg["D"], cfg["H"], cfg["L"], cfg["ncores"]
    nsh, nshp = cfg["nsh"], cfg["nshp"]
    x = np.asarray(x, np.float32)
    calls, gidx, pidx, seg_cols = build_schedule(scatter_idx, node_idx, cfg)
    H0, H1, wlo = cfg["H0"], cfg["H1"], cfg["wlo"]
    xga = np.zeros((wlo, D), bf)
    xgb = np.zeros((N + 4 - wlo, D), bf)
    xb = x.astype(bf)
    for k in range(nc_):
        sh = xb[k * nsh:(k + 1) * nsh]
        xga[2 + k * H0:2 + (k + 1) * H0] = sh[:H0]
        xgb[k * H1:(k + 1) * H1] = sh[H0:]
    w1f = np.asarray(w1, np.float32).reshape(L * H * D, D).astype(bf)
    w2f = np.asarray(w2, np.float32).reshape(L * H * D, D).astype(bf)
    eps1 = np.full((128, 1), 1.0 + float(np.asarray(eps).reshape(-1)[0]),
                   np.float32)
    in_maps = []
    for k in range(nc_):
        xkT = np.zeros((D, nshp), bf)
        xkT[:, :nsh] = x[k * nsh:(k + 1) * nsh].T.astype(bf)
        in_maps.append({
            "xga": xga, "xgb": xgb, "gidx": gidx[k], "pidx": pidx[k],
            "xkT": xkT,
            "w1f": w1f, "w2f": w2f, "eps1": eps1,
            "ident": np.eye(128, dtype=np.float32),
        })
    return calls, seg_cols, in_maps


def kernel_with_results(x, w1, w2, eps, scatter_idx, node_idx, cfg=None,
                        **run_kwargs):
    cfg = cfg or FULL_CFG
    calls, seg_cols, in_maps = _prep_inputs(
        x, w1, w2, eps, scatter_idx, node_idx, cfg)
    nc = build_program(cfg, calls, seg_cols, nqueues=FULL_NQUEUES,
                       bufs=FULL_BUFS, oacc_bf16=True)

    from concourse.bass_utils import run_bass_kernel_spmd
    res = run_bass_kernel_spmd(nc, in_maps,
                               core_ids=list(range(cfg["ncores"])),
                               **run_kwargs)
    outs = [res.results[k]["out"][:cfg["nsh"]] for k in range(cfg["ncores"])]
    return np.concatenate(outs, axis=0).astype(np.float32), res


def kernel(x, w1, w2, eps, scatter_idx, node_idx):
    out, _ = kernel_with_results(x, w1, w2, eps, scatter_idx, node_idx)
    return out

